# revision 14
# baseline (speedup 1.0000x reference)
"""CapsuleLayer dynamic-routing kernel for Trainium2 (8 NeuronCores).

Problem: inputs [B=32, I=2048, J=16], W [N=64, I=2048, D=32, J=16], routings=3.
  inputs_hat[b,n,i,d] = sum_j inputs[b,i,j] * W[n,i,d,j]
  3 rounds of routing (softmax over n, weighted sum over i, squash over d).

Strategy: shard the input-capsule axis I across the 8 cores (I_loc=256).
Each core recomputes its ihat shard from W each round (W streamed from HBM
in bf16, fp8 for round 0; ihat never hits DRAM), keeps its b-state
[*, n, i_loc] in SBUF, and the only cross-core data is the [B, N, D]
partial sum s, AllReduced once per round (bf16 for rounds 0-1, fp32 for
the output round).

Per group of 4 input capsules i (free layout (d,n): d OUTER, n INNER):
  PE:  block-diag K=64 matmuls stream W once ->
       H PSUM [128=(4i,32b), 4x512 quarter tiles]
  SC:  stage H -> SBUF bf16 (hsb)
  DVE: tmpv = hsb*vb (dense bf16 2x mode)
  PE:  t16 = d-halves folded via identity-matmul PSUM accumulation
  SC:  stage t16 -> SBUF bf16
  DVE: t8/t4/t2/y dense tree adds ; b += y ; e = exp(b) (SC) ;
       e_rep8 = e replicated 8x (doubling copies) ; sel' = sel*(1/sum e)
  DVE: tmp2 = hsb * e_rep8 (3D quarters, inner-512 keeps 2x mode)
  PE:  s_psum += sel'.T @ tmp2  (folds softmax denom, partition groups,
       AND the sum over i into one accumulating matmul chain)
GpSimd does no large SBUF ops: concurrent GpSimd/DVE SBUF traffic
serializes both engines on this silicon (measured).
"""

import sys

for p in ("/opt/trn_rl_repo",):
    if p not in sys.path:
        sys.path.insert(0, p)

import ml_dtypes
import numpy as np

import concourse.bacc as bacc
import concourse.mybir as mybir
import concourse.tile as tile
from concourse.bass_utils import run_bass_kernel_spmd

# problem constants (hardcoded per harness contract)
B, N, I, D, J = 32, 64, 2048, 32, 16
R = 3  # routings
CORES = 8
I_LOC = I // CORES  # 256
ND = N * D  # 2048
EPS = 1e-7

F32 = mybir.dt.float32
BF16 = mybir.dt.bfloat16
FP8 = mybir.dt.float8e4
FX = mybir.AxisListType.X
ADD = mybir.AluOpType.add
ACT = mybir.ActivationFunctionType

GROUPS = I_LOC // 4  # 64 groups of 4 capsules per round
HF = ND // 2  # 1024
QF = ND // 4  # 512


def _squash_fast(nc, vbpool, sp, kp, s4, eps_ap):
    """bf16 squash for rounds 0-1: s4 [128, 2048] (d,n) bf16 ->
    vb [128, 2048] bf16.  sq computed via bf16 square + dense tree."""
    s2 = kp.tile([128, ND], BF16, tag="sq_s2", bufs=1)
    nc.scalar.square(s2[:], s4[:])
    u16 = kp.tile([128, HF], BF16, tag="sq_u16", bufs=1)
    nc.vector.tensor_add(u16[:], s2[:, :HF], s2[:, HF:])
    u8 = kp.tile([128, QF], BF16, tag="sq_u8", bufs=1)
    nc.vector.tensor_add(u8[:], u16[:, :QF], u16[:, QF:])
    u4 = kp.tile([128, QF // 2], BF16, tag="sq_u4", bufs=1)
    nc.vector.tensor_add(u4[:], u8[:, :QF // 2], u8[:, QF // 2:])
    u2 = kp.tile([128, QF // 4], BF16, tag="sq_u2", bufs=1)
    nc.vector.tensor_add(u2[:], u4[:, :QF // 4], u4[:, QF // 4:])
    sq = sp.tile([128, N], F32, tag="sq_sq")
    nc.vector.tensor_add(sq[:], u2[:, :N], u2[:, N:])
    # t = sqrt(sq + eps); q1 = 1 + sq
    t = sp.tile([128, N], F32, tag="sq_t")
    nc.scalar.activation(t[:], sq[:], ACT.Sqrt, bias=eps_ap)
    q1 = sp.tile([128, N], F32, tag="sq_q1")
    nc.scalar.activation(q1[:], sq[:], ACT.Identity, bias=1.0)
    den = sp.tile([128, N], F32, tag="sq_den")
    nc.vector.tensor_mul(den[:], q1[:], t[:])
    rs = sp.tile([128, N], F32, tag="sq_rs")
    nc.vector.reciprocal(rs[:], den[:])
    # scale replicated to 512 for the cheap 3D mul
    scr = sp.tile([128, QF], BF16, tag="sq_scr")
    nc.vector.tensor_mul(scr[:, :N], sq[:], rs[:])
    nc.vector.tensor_copy(scr[:, N:2 * N], scr[:, :N])
    nc.vector.tensor_copy(scr[:, 2 * N:4 * N], scr[:, :2 * N])
    nc.vector.tensor_copy(scr[:, 4 * N:], scr[:, :4 * N])
    vb = vbpool.tile([128, ND], BF16, tag="sq_vb")
    nc.vector.tensor_mul(
        vb[:].rearrange("p (q f) -> p q f", q=4),
        s4[:].rearrange("p (q f) -> p q f", q=4),
        scr[:, None, :].broadcast_to([128, 4, QF]),
    )
    return vb


def build_kernel():
    nc = bacc.Bacc("TRN2", target_bir_lowering=False, debug=False)

    # x: [(i j), b] ; w: [(i j), (d n)] with w[(i,j),(d,n)] = W[n, i, d, j]
    # (d OUTER, n INNER in the free dim). fp8 copies for round 0.
    xth = nc.dram_tensor("xth", [I_LOC * J, B], BF16, kind="ExternalInput")
    wth = nc.dram_tensor("wth", [I_LOC * J, ND], BF16, kind="ExternalInput")
    out = nc.dram_tensor("out", [B, N, D], F32, kind="ExternalOutput")

    # collective bounce buffers (one pair per round); bf16 for r<2.
    # the final (fp32) round is split into two b-halves so the AllReduce
    # pipelines with the final squash.
    s_in = [nc.dram_tensor(f"s_in{r}", [B, ND], BF16 if r < 2 else F32)
            for r in range(R)]
    s_out = [nc.dram_tensor(f"s_out{r}", [B, ND], BF16 if r < 2 else F32,
                            addr_space="Shared")
             for r in range(R)]

    with tile.TileContext(nc) as tc:
        with (
            tc.tile_pool(name="persist", bufs=1) as pp,
            tc.tile_pool(name="wsbp", bufs=4) as wsbp,   # round-0 W chunks
            tc.tile_pool(name="wgp", bufs=6) as wgp,     # group W tiles
            tc.tile_pool(name="vbp", bufs=2) as vbp,
            tc.tile_pool(name="work", bufs=2) as kp,
            tc.tile_pool(name="t2p", bufs=2) as t2p,     # tmp2 (fold input)
            tc.tile_pool(name="hsbp", bufs=6) as hsbp,   # staged H bf16
            tc.tile_pool(name="tvp", bufs=2) as tvp,     # tmpv + tree
            tc.tile_pool(name="s4p", bufs=2) as s4p,
            tc.tile_pool(name="pbig", bufs=1) as pbig,
            tc.tile_pool(name="small", bufs=3) as sp,
            tc.tile_pool(name="psum", bufs=2, space="PSUM") as psp,
            tc.tile_pool(name="psumT", bufs=2, space="PSUM") as pst,
            tc.tile_pool(name="psumB", bufs=1, space="PSUM") as psB,
        ):
            # ---- resident tiles ----
            # round-0 stationary: [128=(8i,16j), 32 chunks, B]
            xsb = pp.tile([128, I_LOC * J // 128, B], BF16, tag="xsb")
            nc.sync.dma_start(
                xsb[:], xth[:].rearrange("(k p) b -> p k b", p=128))
            # block-diag stationary: xblk[16c+j, g, 32c+b] = x[b, 4g+c, j]
            xblk = pp.tile([64, GROUPS, 128], BF16, tag="xblk")
            nc.gpsimd.memset(xblk[:], 0.0)
            xv = xth[:].rearrange("(g c j) b -> c j g b", c=4, j=J)
            for c in range(4):
                nc.sync.dma_start(
                    xblk[16 * c:16 * (c + 1), :, 32 * c:32 * (c + 1)], xv[c])

            # routing logits b: [128=(c,b), GROUPS, N]
            bstate = pp.tile([128, GROUPS, N], F32, tag="bstate")
            nc.gpsimd.memset(bstate[:], 0.0)
            eps_t = pp.tile([128, 1], F32, tag="eps")
            nc.gpsimd.memset(eps_t[:], EPS)
            # selector[p, m] = 1.0 if p % 32 == m  (partition-group fold)
            sel_i = pp.tile([128, B], mybir.dt.int32, tag="sel_i")
            nc.gpsimd.iota(sel_i[:], [[1, B]], channel_multiplier=-1)
            nc.vector.tensor_scalar(sel_i[:], sel_i[:], 31, None,
                                    op0=mybir.AluOpType.bitwise_and)
            sel = pp.tile([128, B], BF16, tag="sel")
            nc.vector.tensor_scalar(sel[:], sel_i[:], 0, None,
                                    op0=mybir.AluOpType.is_equal)
            # identity stationary for the PE d-halving accumulation
            id_i = pp.tile([128, 128], mybir.dt.int32, tag="id_i")
            nc.gpsimd.iota(id_i[:], [[1, 128]], channel_multiplier=-1)
            ident = pp.tile([128, 128], BF16, tag="ident")
            nc.vector.tensor_scalar(ident[:], id_i[:], 0, None,
                                    op0=mybir.AluOpType.is_equal)

            # ---------- round 0 (fp8): s0 = (1/N) sum_i ihat ----------
            ps0 = psB.tile([B, ND], F32, tag="pss")
            n_chunks = I_LOC * J // 128  # 32
            for k in range(n_chunks):
                ws = wsbp.tile([128, ND], BF16, tag="wsb")
                nc.sync.dma_start(ws[:], wth[k * 128:(k + 1) * 128, :])
                for q in range(4):
                    nc.tensor.matmul(
                        ps0[:, q * QF:(q + 1) * QF],
                        xsb[:, k, :],
                        ws[:, q * QF:(q + 1) * QF],
                        start=(k == 0),
                        stop=(k == n_chunks - 1),
                    )
            s_loc0 = pbig.tile([B, ND], BF16, tag="s_loc")
            nc.scalar.activation(s_loc0[:], ps0[:], ACT.Copy, scale=1.0 / N)
            nc.sync.dma_start(s_in[0][:], s_loc0[:])
            nc.gpsimd.collective_compute(
                "AllReduce", ADD,
                replica_groups=[list(range(CORES))],
                ins=[s_in[0].ap().opt()], outs=[s_out[0].ap().opt()],
            )
            s4 = s4p.tile([128, ND], BF16, tag="s4")
            for g4 in range(4):
                nc.sync.dma_start(s4[g4 * 32:(g4 + 1) * 32, :], s_out[0][:])
            vb = _squash_fast(nc, vbp, sp, kp, s4, eps_t[:])

            # ---------- rounds 1, 2 ----------
            for r in (1, 2):
                ps_s = psB.tile([B, ND], F32, tag="pss")
                for gp in range(0, GROUPS, 2):
                    # --- per-group: H, staging, tmpv, t8 accumulation ---
                    hsbs = []
                    t8c = tvp.tile([128, 2, QF], BF16, tag="t8c")
                    for gg in range(2):
                        g = gp + gg
                        # W rows for capsules i = 4g..4g+3: [(4i,16j),(d,n)]
                        wg = wgp.tile([64, ND], BF16, tag="wg")
                        nc.sync.dma_start(wg[:], wth[64 * g:64 * (g + 1), :])
                        hsb = hsbp.tile([128, ND], BF16, tag="hsb")
                        for q in range(4):
                            pg = psp.tile([128, QF], F32, tag="pg")
                            nc.tensor.matmul(
                                pg[:], xblk[:, g, :],
                                wg[:, q * QF:(q + 1) * QF],
                                start=True, stop=True,
                            )
                            nc.scalar.copy(hsb[:, q * QF:(q + 1) * QF],
                                           pg[:])
                        hsbs.append(hsb)
                        # tmpv = H*vb (dense bf16, 2x DVE mode)
                        tmpv = tvp.tile([128, ND], BF16, tag="tmpv")
                        nc.vector.tensor_mul(tmpv[:], hsb[:], vb[:])
                        # t8[k,n] = sum_q tmpv[8q+k,n] via one 4-way
                        # identity-matmul PSUM accumulation
                        pt = pst.tile([128, QF], F32, tag="pt")
                        for q in range(4):
                            nc.tensor.matmul(pt[:], ident[:],
                                             tmpv[:, q * QF:(q + 1) * QF],
                                             start=(q == 0), stop=(q == 3),
                                             skip_group_check=True)
                        nc.scalar.copy(t8c[:, gg, :], pt[:])
                    # --- pair-batched tree + b-update (3D APs) ---
                    t4c = tvp.tile([128, 2, QF // 2], BF16, tag="t4c")
                    nc.vector.tensor_add(t4c[:], t8c[:, :, :QF // 2],
                                         t8c[:, :, QF // 2:])
                    t2c = tvp.tile([128, 2, QF // 4], BF16, tag="t2c")
                    nc.vector.tensor_add(t2c[:], t4c[:, :, :QF // 4],
                                         t4c[:, :, QF // 4:])
                    bsl2 = bstate[:, gp:gp + 2, :]
                    nc.vector.tensor_add(bsl2, bsl2, t2c[:, :, :N])
                    nc.vector.tensor_add(bsl2, bsl2, t2c[:, :, N:])
                    # --- softmax pieces (exp per group; rest batched) ---
                    erc = sp.tile([128, 2, QF], BF16, tag="erc")
                    sec = sp.tile([128, 2], F32, tag="sec")
                    for gg in range(2):
                        nc.scalar.activation(erc[:, gg, :N],
                                             bstate[:, gp + gg, :], ACT.Exp,
                                             accum_out=sec[:, gg:gg + 1])
                    nc.vector.tensor_copy(erc[:, :, N:2 * N], erc[:, :, :N])
                    nc.vector.tensor_copy(erc[:, :, 2 * N:4 * N],
                                          erc[:, :, :2 * N])
                    nc.vector.tensor_copy(erc[:, :, 4 * N:],
                                          erc[:, :, :4 * N])
                    rcpc = sp.tile([128, 2], F32, tag="rcpc")
                    nc.vector.reciprocal(rcpc[:], sec[:])
                    # --- per-group: fold stationary, tmp2, fold ---
                    for gg in range(2):
                        g = gp + gg
                        selr = sp.tile([128, B], BF16, tag="selr")
                        nc.vector.tensor_scalar_mul(selr[:], sel[:],
                                                    rcpc[:, gg:gg + 1])
                        tmp2 = t2p.tile([128, ND], BF16, tag="tmp2")
                        nc.vector.tensor_mul(
                            tmp2[:].rearrange("p (q f) -> p q f", q=4),
                            hsbs[gg][:].rearrange("p (q f) -> p q f", q=4),
                            erc[:, gg, None, :].broadcast_to([128, 4, QF]),
                        )
                        for q in range(4):
                            nc.tensor.matmul(
                                ps_s[:, q * QF:(q + 1) * QF],
                                selr[:],
                                tmp2[:, q * QF:(q + 1) * QF],
                                start=(g == 0),
                                stop=(g == GROUPS - 1),
                                skip_group_check=True,
                            )

                if r < 2:
                    s_loc = pbig.tile([B, ND], BF16, tag="s_loc")
                    nc.scalar.copy(s_loc[:], ps_s[:])
                    nc.sync.dma_start(s_in[r][:], s_loc[:])
                else:
                    s_locf = pbig.tile([B, ND], F32, tag="s_locf")
                    nc.scalar.copy(s_locf[:], ps_s[:])
                    nc.sync.dma_start(s_in[r][:], s_locf[:])
                nc.gpsimd.collective_compute(
                    "AllReduce", ADD,
                    replica_groups=[list(range(CORES))],
                    ins=[s_in[r].ap().opt()], outs=[s_out[r].ap().opt()],
                )
                if r < 2:
                    s4 = s4p.tile([128, ND], BF16, tag="s4")
                    for g4 in range(4):
                        nc.sync.dma_start(s4[g4 * 32:(g4 + 1) * 32, :],
                                          s_out[r][:])
                    vb = _squash_fast(nc, vbp, sp, kp, s4, eps_t[:])
                else:
                    # final: squash(s2) rows 0..31, permute (d,n)->(n,d)
                    s4f = s4p.tile([B, ND], F32, tag="s4f", bufs=1)
                    nc.sync.dma_start(s4f[:], s_out[r][:])
                    s2t = kp.tile([B, ND], F32, tag="sq_s2f", bufs=1)
                    nc.scalar.square(s2t[:], s4f[:])
                    v16 = kp.tile([B, HF], F32, tag="sq_v16", bufs=1)
                    nc.vector.tensor_add(v16[:], s2t[:, :HF], s2t[:, HF:])
                    v8 = kp.tile([B, QF], F32, tag="sq_v8", bufs=1)
                    nc.vector.tensor_add(v8[:], v16[:, :QF], v16[:, QF:])
                    v4 = kp.tile([B, QF // 2], F32, tag="sq_v4", bufs=1)
                    nc.vector.tensor_add(v4[:], v8[:, :QF // 2],
                                         v8[:, QF // 2:])
                    v2 = kp.tile([B, QF // 4], F32, tag="sq_v2", bufs=1)
                    nc.vector.tensor_add(v2[:], v4[:, :QF // 4],
                                         v4[:, QF // 4:])
                    sqf = sp.tile([B, N], F32, tag="sqf")
                    nc.vector.tensor_add(sqf[:], v2[:, :N], v2[:, N:])
                    tf = sp.tile([B, N], F32, tag="sq_t")
                    nc.scalar.activation(tf[:], sqf[:], ACT.Sqrt,
                                         bias=eps_t[0:B, :])
                    q1f = sp.tile([B, N], F32, tag="sq_q1")
                    nc.scalar.activation(q1f[:], sqf[:], ACT.Identity,
                                         bias=1.0)
                    denf = sp.tile([B, N], F32, tag="sq_den")
                    nc.vector.tensor_mul(denf[:], q1f[:], tf[:])
                    rsf = sp.tile([B, N], F32, tag="sq_rs")
                    nc.vector.reciprocal(rsf[:], denf[:])
                    scf = sp.tile([B, N], F32, tag="sq_scale")
                    nc.vector.tensor_mul(scf[:], sqf[:], rsf[:])
                    out32 = pbig.tile([B, ND], F32, tag="out32")
                    nc.vector.tensor_mul(
                        out32[:].rearrange("p (n d) -> p n d", d=D),
                        s4f[:].rearrange("p (d n) -> p n d", d=D),
                        scf[:, :, None].broadcast_to([B, N, D]),
                    )
                    nc.sync.dma_start(
                        out[:].rearrange("b n d -> b (n d)"), out32[:])

    nc.compile()
    return nc


_NC_CACHE = {}


def _get_nc():
    if "nc" not in _NC_CACHE:
        _NC_CACHE["nc"] = build_kernel()
    return _NC_CACHE["nc"]


def _make_in_maps(inputs, W):
    inputs = np.ascontiguousarray(np.asarray(inputs, dtype=np.float32))
    W = np.ascontiguousarray(np.asarray(W, dtype=np.float32))
    assert inputs.shape == (B, I, J) and W.shape == (N, I, D, J)
    in_maps = []
    for c in range(CORES):
        sl = slice(c * I_LOC, (c + 1) * I_LOC)
        # xt: [(i j), b]
        x_t = inputs[:, sl, :].transpose(1, 2, 0).reshape(I_LOC * J, B)
        # w2: [(i j), (d n)] ; w2[(i,j),(d,n)] = W[n, i, d, j]
        w_t = W[:, sl, :, :].transpose(1, 3, 2, 0).reshape(I_LOC * J, ND)
        in_maps.append({
            "xth": np.ascontiguousarray(x_t.astype(ml_dtypes.bfloat16)),
            "wth": np.ascontiguousarray(w_t.astype(ml_dtypes.bfloat16)),
        })
    return in_maps


def _ensure_ntff_hook():
    """Register the axon NTFF profile hook if the image's antenv lacks it."""
    import types

    try:
        import antenv.axon_hooks  # noqa: F401
        return
    except ImportError:
        pass
    import antenv

    if "/root/.axon_site" not in sys.path:
        sys.path.insert(0, "/root/.axon_site")
    from trn_agent_boot.trn_boot import _ntff_profile_via_ctypes

    hook = {"h": _ntff_profile_via_ctypes("/opt/axon/libaxon_pjrt.so")}
    mod = types.ModuleType("antenv.axon_hooks")
    mod.get_axon_ntff_profile_hook = lambda: hook["h"]
    mod.set_axon_ntff_profile_hook = lambda h: hook.__setitem__("h", h)
    sys.modules["antenv.axon_hooks"] = mod
    antenv.axon_hooks = mod


def run(inputs, W, trace=False):
    nc = _get_nc()
    if trace:
        _ensure_ntff_hook()
        # zero-egress container: skip the artifact upload, keep files local
        import concourse.bass_utils as bu
        bu.upload_artifacts = lambda d: d
    res = run_bass_kernel_spmd(
        nc, _make_in_maps(inputs, W), core_ids=list(range(CORES)),
        trace=trace,
    )
    return res.results[0]["out"].reshape(B, N, D), res


def kernel(inputs, W, routings=R, **_unused):
    assert int(routings) == R
    out, _ = run(inputs, W, trace=False)
    return out


# revision 15
# speedup vs baseline: 1.1313x; 1.1313x over previous
"""CapsuleLayer dynamic-routing kernel for Trainium2 (8 NeuronCores).

Problem: inputs [B=32, I=2048, J=16], W [N=64, I=2048, D=32, J=16], routings=3.
  inputs_hat[b,n,i,d] = sum_j inputs[b,i,j] * W[n,i,d,j]
  3 rounds of routing (softmax over n, weighted sum over i, squash over d).

Strategy: shard the input-capsule axis I across the 8 cores (I_loc=256).
Each core recomputes its ihat shard from W each round (W streamed from HBM
in bf16, fp8 for round 0; ihat never hits DRAM), keeps its b-state
[*, n, i_loc] in SBUF, and the only cross-core data is the [B, N, D]
partial sum s, AllReduced once per round (bf16 for rounds 0-1, fp32 for
the output round).

Per group of 4 input capsules i (free layout (d,n): d OUTER, n INNER):
  PE:  block-diag K=64 matmuls stream W once ->
       H PSUM [128=(4i,32b), 4x512 quarter tiles]
  SC:  stage H -> SBUF bf16 (hsb)
  DVE: tmpv = hsb*vb (dense bf16 2x mode)
  PE:  t16 = d-halves folded via identity-matmul PSUM accumulation
  SC:  stage t16 -> SBUF bf16
  DVE: t8/t4/t2/y dense tree adds ; b += y ; e = exp(b) (SC) ;
       e_rep8 = e replicated 8x (doubling copies) ; sel' = sel*(1/sum e)
  DVE: tmp2 = hsb * e_rep8 (3D quarters, inner-512 keeps 2x mode)
  PE:  s_psum += sel'.T @ tmp2  (folds softmax denom, partition groups,
       AND the sum over i into one accumulating matmul chain)
GpSimd does no large SBUF ops: concurrent GpSimd/DVE SBUF traffic
serializes both engines on this silicon (measured).
"""

import sys

for p in ("/opt/trn_rl_repo",):
    if p not in sys.path:
        sys.path.insert(0, p)

import ml_dtypes
import numpy as np

import concourse.bacc as bacc
import concourse.mybir as mybir
import concourse.tile as tile
from concourse.bass_utils import run_bass_kernel_spmd

# problem constants (hardcoded per harness contract)
B, N, I, D, J = 32, 64, 2048, 32, 16
R = 3  # routings
CORES = 8
I_LOC = I // CORES  # 256
ND = N * D  # 2048
EPS = 1e-7

F32 = mybir.dt.float32
BF16 = mybir.dt.bfloat16
FP8 = mybir.dt.float8e4
FX = mybir.AxisListType.X
ADD = mybir.AluOpType.add
ACT = mybir.ActivationFunctionType

GROUPS = I_LOC // 4  # 64 groups of 4 capsules per round
HF = ND // 2  # 1024
QF = ND // 4  # 512


def _squash_fast(nc, vbpool, sp, kp, s4, eps_ap):
    """bf16 squash for rounds 0-1: s4 [128, 2048] (d,n) bf16 ->
    vb [128, 2048] bf16.  sq computed via bf16 square + dense tree."""
    s2 = kp.tile([128, ND], BF16, tag="sq_s2", bufs=1)
    nc.scalar.square(s2[:], s4[:])
    u16 = kp.tile([128, HF], BF16, tag="sq_u16", bufs=1)
    nc.vector.tensor_add(u16[:], s2[:, :HF], s2[:, HF:])
    u8 = kp.tile([128, QF], BF16, tag="sq_u8", bufs=1)
    nc.vector.tensor_add(u8[:], u16[:, :QF], u16[:, QF:])
    u4 = kp.tile([128, QF // 2], BF16, tag="sq_u4", bufs=1)
    nc.vector.tensor_add(u4[:], u8[:, :QF // 2], u8[:, QF // 2:])
    u2 = kp.tile([128, QF // 4], BF16, tag="sq_u2", bufs=1)
    nc.vector.tensor_add(u2[:], u4[:, :QF // 4], u4[:, QF // 4:])
    sq = sp.tile([128, N], F32, tag="sq_sq")
    nc.vector.tensor_add(sq[:], u2[:, :N], u2[:, N:])
    # t = sqrt(sq + eps); q1 = 1 + sq
    t = sp.tile([128, N], F32, tag="sq_t")
    nc.scalar.activation(t[:], sq[:], ACT.Sqrt, bias=eps_ap)
    q1 = sp.tile([128, N], F32, tag="sq_q1")
    nc.scalar.activation(q1[:], sq[:], ACT.Identity, bias=1.0)
    den = sp.tile([128, N], F32, tag="sq_den")
    nc.vector.tensor_mul(den[:], q1[:], t[:])
    rs = sp.tile([128, N], F32, tag="sq_rs")
    nc.vector.reciprocal(rs[:], den[:])
    # scale replicated to 512 for the cheap 3D mul
    scr = sp.tile([128, QF], BF16, tag="sq_scr")
    nc.vector.tensor_mul(scr[:, :N], sq[:], rs[:])
    nc.vector.tensor_copy(scr[:, N:2 * N], scr[:, :N])
    nc.vector.tensor_copy(scr[:, 2 * N:4 * N], scr[:, :2 * N])
    nc.vector.tensor_copy(scr[:, 4 * N:], scr[:, :4 * N])
    vb = vbpool.tile([128, ND], BF16, tag="sq_vb")
    nc.vector.tensor_mul(
        vb[:].rearrange("p (q f) -> p q f", q=4),
        s4[:].rearrange("p (q f) -> p q f", q=4),
        scr[:, None, :].broadcast_to([128, 4, QF]),
    )
    return vb


def build_kernel():
    nc = bacc.Bacc("TRN2", target_bir_lowering=False, debug=False)

    # x: [(i j), b] ; w: [(i j), (d n)] with w[(i,j),(d,n)] = W[n, i, d, j]
    # (d OUTER, n INNER in the free dim). fp8 copies for round 0.
    xth = nc.dram_tensor("xth", [I_LOC * J, B], BF16, kind="ExternalInput")
    wth = nc.dram_tensor("wth", [I_LOC * J, ND], BF16, kind="ExternalInput")
    out = nc.dram_tensor("out", [B, N, D], F32, kind="ExternalOutput")

    # collective bounce buffers (one pair per round); bf16 for r<2.
    # the final (fp32) round is split into two b-halves so the AllReduce
    # pipelines with the final squash.
    s_in = [nc.dram_tensor(f"s_in{r}", [B, ND], BF16 if r < 2 else F32)
            for r in range(R)]
    s_out = [nc.dram_tensor(f"s_out{r}", [B, ND], BF16 if r < 2 else F32,
                            addr_space="Shared")
             for r in range(R)]

    with tile.TileContext(nc) as tc:
        with (
            tc.tile_pool(name="persist", bufs=1) as pp,
            tc.tile_pool(name="wsbp", bufs=4) as wsbp,   # round-0 W chunks
            tc.tile_pool(name="wgp", bufs=6) as wgp,     # group W tiles
            tc.tile_pool(name="vbp", bufs=2) as vbp,
            tc.tile_pool(name="work", bufs=2) as kp,
            tc.tile_pool(name="t2p", bufs=2) as t2p,     # tmp2 (fold input)
            tc.tile_pool(name="hsbp", bufs=6) as hsbp,   # staged H bf16
            tc.tile_pool(name="tvp", bufs=2) as tvp,     # tmpv + tree
            tc.tile_pool(name="s4p", bufs=2) as s4p,
            tc.tile_pool(name="pbig", bufs=1) as pbig,
            tc.tile_pool(name="small", bufs=3) as sp,
            tc.tile_pool(name="psum", bufs=2, space="PSUM") as psp,
            tc.tile_pool(name="psumT", bufs=2, space="PSUM") as pst,
            tc.tile_pool(name="psumB", bufs=1, space="PSUM") as psB,
        ):
            # ---- resident tiles ----
            # round-0 stationary: [128=(8i,16j), 32 chunks, B]
            xsb = pp.tile([128, I_LOC * J // 128, B], BF16, tag="xsb")
            nc.sync.dma_start(
                xsb[:], xth[:].rearrange("(k p) b -> p k b", p=128))
            # block-diag stationary: xblk[16c+j, g, 32c+b] = x[b, 4g+c, j]
            xblk = pp.tile([64, GROUPS, 128], BF16, tag="xblk")
            nc.gpsimd.memset(xblk[:], 0.0)
            xv = xth[:].rearrange("(g c j) b -> c j g b", c=4, j=J)
            for c in range(4):
                nc.sync.dma_start(
                    xblk[16 * c:16 * (c + 1), :, 32 * c:32 * (c + 1)], xv[c])

            # routing logits b: [128=(c,b), GROUPS, N]
            bstate = pp.tile([128, GROUPS, N], F32, tag="bstate")
            nc.gpsimd.memset(bstate[:], 0.0)
            eps_t = pp.tile([128, 1], F32, tag="eps")
            nc.gpsimd.memset(eps_t[:], EPS)
            # selector[p, m] = 1.0 if p % 32 == m  (partition-group fold)
            sel_i = pp.tile([128, B], mybir.dt.int32, tag="sel_i")
            nc.gpsimd.iota(sel_i[:], [[1, B]], channel_multiplier=-1)
            nc.vector.tensor_scalar(sel_i[:], sel_i[:], 31, None,
                                    op0=mybir.AluOpType.bitwise_and)
            sel = pp.tile([128, B], BF16, tag="sel")
            nc.vector.tensor_scalar(sel[:], sel_i[:], 0, None,
                                    op0=mybir.AluOpType.is_equal)
            # identity stationary for the PE d-halving accumulation
            id_i = pp.tile([128, 128], mybir.dt.int32, tag="id_i")
            nc.gpsimd.iota(id_i[:], [[1, 128]], channel_multiplier=-1)
            ident = pp.tile([128, 128], BF16, tag="ident")
            nc.vector.tensor_scalar(ident[:], id_i[:], 0, None,
                                    op0=mybir.AluOpType.is_equal)

            # ---------- round 0 (fp8): s0 = (1/N) sum_i ihat ----------
            ps0 = psB.tile([B, ND], F32, tag="pss")
            n_chunks = I_LOC * J // 128  # 32
            for k in range(n_chunks):
                ws = wsbp.tile([128, ND], BF16, tag="wsb")
                nc.sync.dma_start(ws[:], wth[k * 128:(k + 1) * 128, :])
                for q in range(4):
                    nc.tensor.matmul(
                        ps0[:, q * QF:(q + 1) * QF],
                        xsb[:, k, :],
                        ws[:, q * QF:(q + 1) * QF],
                        start=(k == 0),
                        stop=(k == n_chunks - 1),
                    )
            s_loc0 = pbig.tile([B, ND], BF16, tag="s_loc")
            nc.scalar.activation(s_loc0[:], ps0[:], ACT.Copy, scale=1.0 / N)
            nc.sync.dma_start(s_in[0][:], s_loc0[:])
            nc.gpsimd.collective_compute(
                "AllReduce", ADD,
                replica_groups=[list(range(CORES))],
                ins=[s_in[0].ap().opt()], outs=[s_out[0].ap().opt()],
            )
            s4 = s4p.tile([128, ND], BF16, tag="s4")
            for g4 in range(4):
                nc.sync.dma_start(s4[g4 * 32:(g4 + 1) * 32, :], s_out[0][:])
            vb = _squash_fast(nc, vbp, sp, kp, s4, eps_t[:])

            # ---------- rounds 1, 2 ----------
            for r in (1, 2):
                ps_s = psB.tile([B, ND], F32, tag="pss")
                for g in range(GROUPS):
                    # W rows for capsules i = 4g..4g+3 : [(4i,16j), (d,n)]
                    wg = wgp.tile([64, ND], BF16, tag="wg")
                    nc.sync.dma_start(wg[:], wth[64 * g:64 * (g + 1), :])
                    # H via block-diag K=64 matmuls -> 4 PSUM quarters
                    hsb = hsbp.tile([128, ND], BF16, tag="hsb")
                    for q in range(4):
                        pg = psp.tile([128, QF], F32, tag="pg")
                        nc.tensor.matmul(
                            pg[:], xblk[:, g, :],
                            wg[:, q * QF:(q + 1) * QF],
                            start=True, stop=True,
                        )
                        # stage quarter -> SBUF bf16 (ScalarE)
                        nc.scalar.copy(hsb[:, q * QF:(q + 1) * QF], pg[:])
                    # tmpv = H*vb (dense bf16, 2x DVE mode)
                    tmpv = tvp.tile([128, ND], BF16, tag="tmpv")
                    nc.vector.tensor_mul(tmpv[:], hsb[:], vb[:])
                    # t8[k,n] = sum_q tmpv[8q+k, n] via one 4-way
                    # identity-matmul PSUM accumulation (any d-grouping is
                    # valid -- the tree sums all d eventually)
                    pt = pst.tile([128, QF], F32, tag="pt")
                    for q in range(4):
                        nc.tensor.matmul(pt[:], ident[:],
                                         tmpv[:, q * QF:(q + 1) * QF],
                                         start=(q == 0), stop=(q == 3),
                                         skip_group_check=True)
                    t8 = tvp.tile([128, QF], BF16, tag="t8")
                    nc.scalar.copy(t8[:], pt[:])
                    t4 = tvp.tile([128, QF // 2], BF16, tag="t4")
                    nc.vector.tensor_add(t4[:], t8[:, :QF // 2],
                                         t8[:, QF // 2:])
                    t2 = tvp.tile([128, QF // 4], BF16, tag="t2")
                    nc.vector.tensor_add(t2[:], t4[:, :QF // 4],
                                         t4[:, QF // 4:])
                    # b += both t2 halves (no separate y tile)
                    bsl = bstate[:, g, :]
                    nc.vector.tensor_add(bsl, bsl, t2[:, :N])
                    nc.vector.tensor_add(bsl, bsl, t2[:, N:])
                    er = sp.tile([128, QF], BF16, tag="er")
                    se = sp.tile([128, 1], F32, tag="se")
                    nc.scalar.activation(er[:, :N], bsl, ACT.Exp,
                                         accum_out=se[:])
                    nc.vector.tensor_copy(er[:, N:2 * N], er[:, :N])
                    nc.vector.tensor_copy(er[:, 2 * N:4 * N], er[:, :2 * N])
                    nc.vector.tensor_copy(er[:, 4 * N:], er[:, :4 * N])
                    rcp = sp.tile([128, 1], F32, tag="rcp")
                    nc.vector.reciprocal(rcp[:], se[:])
                    # fold stationary: sel' = sel * (1/sum e)  (r folded in)
                    selr = sp.tile([128, B], BF16, tag="selr")
                    nc.vector.tensor_scalar_mul(selr[:], sel[:], rcp[:])
                    # tmp2 = e * H  (3D quarters; inner 512 keeps 2x mode)
                    tmp2 = t2p.tile([128, ND], BF16, tag="tmp2")
                    nc.vector.tensor_mul(
                        tmp2[:].rearrange("p (q f) -> p q f", q=4),
                        hsb[:].rearrange("p (q f) -> p q f", q=4),
                        er[:, None, :].broadcast_to([128, 4, QF]),
                    )
                    # fold into s accumulator
                    for q in range(4):
                        nc.tensor.matmul(
                            ps_s[:, q * QF:(q + 1) * QF],
                            selr[:],
                            tmp2[:, q * QF:(q + 1) * QF],
                            start=(g == 0),
                            stop=(g == GROUPS - 1),
                            skip_group_check=True,
                        )

                if r < 2:
                    s_loc = pbig.tile([B, ND], BF16, tag="s_loc")
                    nc.scalar.copy(s_loc[:], ps_s[:])
                    nc.sync.dma_start(s_in[r][:], s_loc[:])
                else:
                    s_locf = pbig.tile([B, ND], F32, tag="s_locf")
                    nc.scalar.copy(s_locf[:], ps_s[:])
                    nc.sync.dma_start(s_in[r][:], s_locf[:])
                nc.gpsimd.collective_compute(
                    "AllReduce", ADD,
                    replica_groups=[list(range(CORES))],
                    ins=[s_in[r].ap().opt()], outs=[s_out[r].ap().opt()],
                )
                if r < 2:
                    s4 = s4p.tile([128, ND], BF16, tag="s4")
                    for g4 in range(4):
                        nc.sync.dma_start(s4[g4 * 32:(g4 + 1) * 32, :],
                                          s_out[r][:])
                    vb = _squash_fast(nc, vbp, sp, kp, s4, eps_t[:])
                else:
                    # final: squash(s2) rows 0..31, permute (d,n)->(n,d)
                    s4f = s4p.tile([B, ND], F32, tag="s4f", bufs=1)
                    nc.sync.dma_start(s4f[:], s_out[r][:])
                    s2t = kp.tile([B, ND], F32, tag="sq_s2f", bufs=1)
                    nc.scalar.square(s2t[:], s4f[:])
                    v16 = kp.tile([B, HF], F32, tag="sq_v16", bufs=1)
                    nc.vector.tensor_add(v16[:], s2t[:, :HF], s2t[:, HF:])
                    v8 = kp.tile([B, QF], F32, tag="sq_v8", bufs=1)
                    nc.vector.tensor_add(v8[:], v16[:, :QF], v16[:, QF:])
                    v4 = kp.tile([B, QF // 2], F32, tag="sq_v4", bufs=1)
                    nc.vector.tensor_add(v4[:], v8[:, :QF // 2],
                                         v8[:, QF // 2:])
                    v2 = kp.tile([B, QF // 4], F32, tag="sq_v2", bufs=1)
                    nc.vector.tensor_add(v2[:], v4[:, :QF // 4],
                                         v4[:, QF // 4:])
                    sqf = sp.tile([B, N], F32, tag="sqf")
                    nc.vector.tensor_add(sqf[:], v2[:, :N], v2[:, N:])
                    tf = sp.tile([B, N], F32, tag="sq_t")
                    nc.scalar.activation(tf[:], sqf[:], ACT.Sqrt,
                                         bias=eps_t[0:B, :])
                    q1f = sp.tile([B, N], F32, tag="sq_q1")
                    nc.scalar.activation(q1f[:], sqf[:], ACT.Identity,
                                         bias=1.0)
                    denf = sp.tile([B, N], F32, tag="sq_den")
                    nc.vector.tensor_mul(denf[:], q1f[:], tf[:])
                    rsf = sp.tile([B, N], F32, tag="sq_rs")
                    nc.vector.reciprocal(rsf[:], denf[:])
                    scf = sp.tile([B, N], F32, tag="sq_scale")
                    nc.vector.tensor_mul(scf[:], sqf[:], rsf[:])
                    out32 = pbig.tile([B, ND], F32, tag="out32")
                    nc.vector.tensor_mul(
                        out32[:].rearrange("p (n d) -> p n d", d=D),
                        s4f[:].rearrange("p (d n) -> p n d", d=D),
                        scf[:, :, None].broadcast_to([B, N, D]),
                    )
                    nc.sync.dma_start(
                        out[:].rearrange("b n d -> b (n d)"), out32[:])

    nc.compile()
    return nc


_NC_CACHE = {}


def _get_nc():
    if "nc" not in _NC_CACHE:
        _NC_CACHE["nc"] = build_kernel()
    return _NC_CACHE["nc"]


def _make_in_maps(inputs, W):
    inputs = np.ascontiguousarray(np.asarray(inputs, dtype=np.float32))
    W = np.ascontiguousarray(np.asarray(W, dtype=np.float32))
    assert inputs.shape == (B, I, J) and W.shape == (N, I, D, J)
    in_maps = []
    for c in range(CORES):
        sl = slice(c * I_LOC, (c + 1) * I_LOC)
        # xt: [(i j), b]
        x_t = inputs[:, sl, :].transpose(1, 2, 0).reshape(I_LOC * J, B)
        # w2: [(i j), (d n)] ; w2[(i,j),(d,n)] = W[n, i, d, j]
        w_t = W[:, sl, :, :].transpose(1, 3, 2, 0).reshape(I_LOC * J, ND)
        in_maps.append({
            "xth": np.ascontiguousarray(x_t.astype(ml_dtypes.bfloat16)),
            "wth": np.ascontiguousarray(w_t.astype(ml_dtypes.bfloat16)),
        })
    return in_maps


def _ensure_ntff_hook():
    """Register the axon NTFF profile hook if the image's antenv lacks it."""
    import types

    try:
        import antenv.axon_hooks  # noqa: F401
        return
    except ImportError:
        pass
    import antenv

    if "/root/.axon_site" not in sys.path:
        sys.path.insert(0, "/root/.axon_site")
    from trn_agent_boot.trn_boot import _ntff_profile_via_ctypes

    hook = {"h": _ntff_profile_via_ctypes("/opt/axon/libaxon_pjrt.so")}
    mod = types.ModuleType("antenv.axon_hooks")
    mod.get_axon_ntff_profile_hook = lambda: hook["h"]
    mod.set_axon_ntff_profile_hook = lambda h: hook.__setitem__("h", h)
    sys.modules["antenv.axon_hooks"] = mod
    antenv.axon_hooks = mod


def run(inputs, W, trace=False):
    nc = _get_nc()
    if trace:
        _ensure_ntff_hook()
        # zero-egress container: skip the artifact upload, keep files local
        import concourse.bass_utils as bu
        bu.upload_artifacts = lambda d: d
    res = run_bass_kernel_spmd(
        nc, _make_in_maps(inputs, W), core_ids=list(range(CORES)),
        trace=trace,
    )
    return res.results[0]["out"].reshape(B, N, D), res


def kernel(inputs, W, routings=R, **_unused):
    assert int(routings) == R
    out, _ = run(inputs, W, trace=False)
    return out
